# revision 30
# baseline (speedup 1.0000x reference)
"""Qwen3-style GQA attention (B=1, S=2048, DM=2048, H=16, KV=4, D=128) on 8 TRN2 cores.

Sharding: tensor-parallel over heads. Core c computes Q heads {2c, 2c+1} and
KV head c//2 end-to-end, then a partial output hs_part = gated_local @ Wo_rows.
Host sums the 8 partials.

Precision scheme: all matmuls run single-term fp32r (11-bit mantissa operands,
exact products, fp32 PSUM accumulation) at full PE rate; measured end-to-end
error ~1e-2 against the fp32 reference (tolerance 2e-2), dominated by fp32r
rounding of q/k at near-tie softmax logits. RMS-norm scale path: fp32r
sum-of-squares, fp32 sqrt, ~18-bit fast reciprocal, exact fp32 ones-broadcast
matmul. Post-softmax probabilities, diag(1/Z), and V run bf16 (linear ~4e-3
error; bf16 transposes run 1 cycle/row vs 4 for f32r at free-dim 128). Norm
weights fold into Wq/Wk host-side; sum-of-squares uses a 1/w^2 stationary
vector so the rms stays exact.

Structure: one fused projection pass streams hsT chunks once and computes all
six outputs; RMSNorm + RoPE + bf16-copy fused per 512-slice, deferred one
slice to hide latency. RoPE's rotate-half is an exact partition-offset
SBUF->SBUF DMA (sign folded into a host-prepared signed sin table). DMAs are
batched 4-contraction-chunks per transfer and spread over both HWDGE queues
(weights/hs/output on sync, cos/sin/rot on scalar) — a dma_start costs ~0.7us
of issuing-engine time, so issue count matters. Attention runs both heads
interleaved per q-chunk; a bf16 score pre-pass provides the softmax max so the
fp32r scores go matmul->exp with no reduce in between (PSUM banks free
immediately, PE stays warm). P^T for AV comes from a bf16 matmul against
diag(1/Z), fusing normalization into the transpose; PSUM evacuation copies
alternate between the vector and scalar engines. Wo partial matmuls run inside
the q-chunk loop to fill PE bubbles, ping-ponging two PSUM banks so the
PSUM->SBUF copy of one output tile overlaps the next tile's matmuls.
"""

import numpy as np

S = 2048
DM = 2048
D = 128
HPC = 2           # q heads per core
NCORES = 8
SCALING = float(D) ** 0.5
EPS = 1e-6
P = 128
KCH = DM // P     # 16 contraction chunks for projections
NQB = S // P      # 16 q blocks
NSC = S // 512    # 4 seq chunks of 512

_cache = {}


def _round_fp32r(x):
    x = np.ascontiguousarray(x, dtype=np.float32)
    b = x.view(np.uint32).astype(np.uint64)
    lsb = (b >> 12) & 1
    r = (b + 0x7FF + lsb) & 0xFFFFF000
    return r.astype(np.uint32).view(np.float32)


def _build_nc():
    import concourse.tile as tile
    from concourse import bacc, mybir

    F32 = mybir.dt.float32
    F32R = mybir.dt.float32r
    BF16 = mybir.dt.bfloat16
    AF = mybir.ActivationFunctionType
    from concourse.alu_op_type import AluOpType as ALU
    AX = mybir.AxisListType.X
    XY = mybir.AxisListType.XY

    nc = bacc.Bacc(None, target_bir_lowering=False, debug=False)

    with nc.allow_low_precision(reason="fp32r/bf16 operands are a deliberate "
                                "precision/speed tradeoff"), \
         tile.TileContext(nc) as tc:
        with tc.tile_pool(name="dram", bufs=1, space="DRAM") as dram:
            hsT = dram.tile([DM, S], F32R, kind="ExternalInput", name="hsT", uniquify=False)
            wq = dram.tile([DM, HPC * P], F32R, kind="ExternalInput", name="wq", uniquify=False)
            wk = dram.tile([DM, P], F32R, kind="ExternalInput", name="wk", uniquify=False)
            wg = dram.tile([DM, HPC * P], F32R, kind="ExternalInput", name="wg", uniquify=False)
            wv = dram.tile([DM, P], F32R, kind="ExternalInput", name="wv", uniquify=False)
            wo = dram.tile([HPC * P, DM], F32R, kind="ExternalInput", name="wo", uniquify=False)
            cosT = dram.tile([P, S], F32, kind="ExternalInput", name="cosT", uniquify=False)
            sinTs = dram.tile([P, S], F32, kind="ExternalInput", name="sinTs", uniquify=False)
            wi2q = dram.tile([P, 1], F32R, kind="ExternalInput", name="wi2q", uniquify=False)
            wi2k = dram.tile([P, 1], F32R, kind="ExternalInput", name="wi2k", uniquify=False)
            identb = dram.tile([P, P], BF16, kind="ExternalInput", name="identb", uniquify=False)
            oner = dram.tile([1, P], F32, kind="ExternalInput", name="oner", uniquify=False)
            triu = dram.tile([P, P], F32, kind="ExternalInput", name="triu", uniquify=False)
            out = dram.tile([S, DM], F32, kind="ExternalOutput", name="out", uniquify=False)

        # persistent SBUF (whole kernel)
        with tc.tile_pool(name="persist", bufs=1) as pers:
            wi2q_sb = pers.tile([P, 1], F32R)
            wi2k_sb = pers.tile([P, 1], F32R)
            identb_sb = pers.tile([P, P], BF16)
            oner_sb = pers.tile([1, P], F32)
            triu_sb = pers.tile([P, P], F32)
            eps_sb = pers.tile([1, 1], F32)
            k_hi = pers.tile([P, S], F32R)
            k_hb = pers.tile([P, S], BF16)
            q_hi = pers.tile([P, HPC, S], F32R)
            q_hb = pers.tile([P, HPC, S], BF16)
            sig_r = pers.tile([P, HPC, S], F32R)    # sigmoid(gate)
            v_r = pers.tile([P, NQB, P], BF16)      # V untransposed (s-major blocks)
            gated_r = pers.tile([P, HPC, S], F32R)

            nc.sync.dma_start(wi2q_sb[:], wi2q[:])
            nc.sync.dma_start(wi2k_sb[:], wi2k[:])
            nc.sync.dma_start(identb_sb[:], identb[:])
            nc.sync.dma_start(oner_sb[:], oner[:])
            nc.sync.dma_start(triu_sb[:], triu[:])
            nc.gpsimd.memset(eps_sb[:], EPS)

            hsTr = hsT.rearrange("(kc p) s -> p kc s", p=P)

            # ====== P1 (fused): all projections + norm + rope + splits ======
            with (
                tc.tile_pool(name="wts", bufs=1) as wpool,
                tc.tile_pool(name="cs", bufs=2) as cspool,
                tc.tile_pool(name="hs1", bufs=2) as hspool,
                tc.tile_pool(name="nsc", bufs=4) as nsc,
                tc.tile_pool(name="nxr", bufs=6) as nxr,
                tc.tile_pool(name="rr1", bufs=2) as rr1,
                tc.tile_pool(name="vts", bufs=1) as vtp,
                tc.tile_pool(name="pqk", bufs=1, space="PSUM") as pqk,
                tc.tile_pool(name="pnm", bufs=1, space="PSUM") as pnm,
            ):
                wq_sb = wpool.tile([P, KCH, HPC * P], F32R)
                wk_sb = wpool.tile([P, KCH, P], F32R)
                wg_sb = wpool.tile([P, KCH, HPC * P], F32R)
                wv_sb = wpool.tile([P, KCH, P], F32R)

                pend = []

                def norm_rope(xr, wvec, xhi, xhb, cos_t, sin_t):
                    sqf = nsc.tile([P, 512], F32R, tag="scr", name="sqf")
                    nc.vector.tensor_mul(sqf[:], xr[:], xr[:])
                    # f32r sumsq matmul; fp32 sqrt + ~18-bit recip; exact
                    # fp32 ones-broadcast matmul
                    ps1 = pnm.tile([1, 512], F32, tag="ps1", name="ps1")
                    nc.tensor.matmul(ps1[:], lhsT=wvec[:], rhs=sqf[:],
                                     start=True, stop=True)
                    sqv = rr1.tile([1, 512], F32, tag="sqv", name="sqv")
                    nc.scalar.activation(sqv[:], ps1[:], AF.Sqrt,
                                         scale=1.0 / D, bias=eps_sb[:])
                    rr = rr1.tile([1, 512], F32, tag="rr", name="rr")
                    nc.vector.reciprocal_approx_fast(rr[:], sqv[:])
                    psb = pnm.tile([P, 512], F32, tag="psb", name="psb")
                    nc.tensor.matmul(psb[:], lhsT=oner_sb[:], rhs=rr[:],
                                     start=True, stop=True)
                    xn = nsc.tile([P, 512], F32, tag="scr", name="xn")
                    nc.vector.tensor_mul(xn[:], xr[:], psb[:])
                    # rotate-half via partition-offset SBUF DMA (exact)
                    rot = nsc.tile([P, 512], F32, tag="scr", name="rot")
                    nc.scalar.dma_start(rot[0:64, :], xn[64:128, :])
                    nc.scalar.dma_start(rot[64:128, :], xn[0:64, :])
                    t2 = nsc.tile([P, 512], F32, tag="scr", name="t2")
                    nc.vector.tensor_mul(t2[:], rot[:], sin_t[:])
                    t1 = nsc.tile([P, 512], F32, tag="scr", name="t1")
                    nc.gpsimd.tensor_mul(t1[:], xn[:], cos_t[:])
                    xf = nsc.tile([P, 512], F32, tag="scr", name="xf")
                    nc.vector.tensor_add(xf[:], t1[:], t2[:])
                    nc.any.tensor_copy(xhi, xf[:])
                    nc.gpsimd.tensor_copy(xhb, xhi.bitcast(F32))

                for sq in range(NSC):
                    s0 = sq * 512
                    sl = slice(s0, s0 + 512)
                    cos_t = cspool.tile([P, 512], F32, tag="cos", name="cos_t")
                    sin_t = cspool.tile([P, 512], F32, tag="sin", name="sin_t")
                    nc.scalar.dma_start(cos_t[:], cosT[:, sl])
                    nc.scalar.dma_start(sin_t[:], sinTs[:, sl])
                    ps_q0 = pqk.tile([P, 512], F32, tag="psq0", name="ps_q0")
                    ps_q1 = pqk.tile([P, 512], F32, tag="psq1", name="ps_q1")
                    ps_k = pqk.tile([P, 512], F32, tag="psk", name="ps_k")
                    ps_g0 = pqk.tile([P, 512], F32, tag="psg0", name="ps_g0")
                    ps_g1 = pqk.tile([P, 512], F32, tag="psg1", name="ps_g1")
                    ps_v = pqk.tile([P, 512], F32, tag="psv", name="ps_v")
                    for k4 in range(4):
                        if sq == 0:
                            # weight DMAs interleave with the hs stream on the
                            # sync queue so the first matmuls start ~5us in
                            ksl = slice(k4 * 4, k4 * 4 + 4)
                            for (dst, src) in (
                                (wq_sb, wq), (wk_sb, wk), (wg_sb, wg), (wv_sb, wv),
                            ):
                                nc.sync.dma_start(
                                    dst[:, ksl, :],
                                    src.rearrange("(kc p) m -> p kc m", p=P)[:, ksl, :])
                        hh = hspool.tile([P, 4, 512], F32R, tag="hh", name="hh")
                        if sq == 0 and k4 == 0:
                            # first hs batch rides the otherwise-idle scalar
                            # queue, in parallel with the first weight batch
                            nc.scalar.dma_start(hh[:], hsTr[:, 0:4, sl])
                        else:
                            nc.sync.dma_start(hh[:], hsTr[:, k4 * 4:k4 * 4 + 4, sl])
                        for kci in range(4):
                            kc = k4 * 4 + kci
                            st = kc == 0
                            sp = kc == KCH - 1
                            hx = hh[:, kci, :]
                            nc.tensor.matmul(ps_q0[:], lhsT=wq_sb[:, kc, 0:P],
                                             rhs=hx, start=st, stop=sp)
                            nc.tensor.matmul(ps_q1[:], lhsT=wq_sb[:, kc, P:2 * P],
                                             rhs=hx, start=st, stop=sp)
                            nc.tensor.matmul(ps_k[:], lhsT=wk_sb[:, kc, :],
                                             rhs=hx, start=st, stop=sp)
                            nc.tensor.matmul(ps_g0[:], lhsT=wg_sb[:, kc, 0:P],
                                             rhs=hx, start=st, stop=sp)
                            nc.tensor.matmul(ps_g1[:], lhsT=wg_sb[:, kc, P:2 * P],
                                             rhs=hx, start=st, stop=sp)
                            nc.tensor.matmul(ps_v[:], lhsT=wv_sb[:, kc, :],
                                             rhs=hx, start=st, stop=sp)
                    # gate: sigmoid straight off PSUM
                    nc.scalar.activation(sig_r[:, 0, sl], ps_g0[:], AF.Sigmoid)
                    nc.scalar.activation(sig_r[:, 1, sl], ps_g1[:], AF.Sigmoid)
                    # V: bf16 copy + transpose into s-major blocks
                    vt = vtp.tile([P, 512], BF16, tag="vt", name="vt")
                    nc.any.tensor_copy(vt[:], ps_v[:])
                    for j in range(4):
                        pst = pqk.tile([P, P], BF16, tag="psg0", name="pst")
                        nc.tensor.transpose(pst[:], vt[:, j * P:(j + 1) * P], identb_sb[:])
                        nc.any.tensor_copy(v_r[:, sq * 4 + j, :], pst[:])
                    # Q/K: copy raw projections out now (frees PSUM); the
                    # norm/rope chain is deferred one sq iteration so the next
                    # projection block hides its PE matmuls' input latency
                    for (psd, wvec, xhi, xhb) in (
                        (ps_q0, wi2q_sb, q_hi[:, 0, sl], q_hb[:, 0, sl]),
                        (ps_q1, wi2q_sb, q_hi[:, 1, sl], q_hb[:, 1, sl]),
                        (ps_k, wi2k_sb, k_hi[:, sl], k_hb[:, sl]),
                    ):
                        xr = nxr.tile([P, 512], F32, tag="xr", name="xr")
                        nc.any.tensor_copy(xr[:], psd[:])
                        pend.append((xr, wvec, xhi, xhb, cos_t, sin_t))
                    if sq > 0:
                        for job in pend[:3]:
                            norm_rope(*job)
                        del pend[:3]
                for job in pend:
                    norm_rope(*job)
                pend.clear()

            # ====== P3: attention, heads interleaved, Wo folded in ======
            with (
                tc.tile_pool(name="mxp", bufs=2, space="PSUM") as mxp,
                tc.tile_pool(name="scp", bufs=3, space="PSUM") as scp,
                tc.tile_pool(name="ptp", bufs=2, space="PSUM") as ptp,
                tc.tile_pool(name="otp", bufs=1, space="PSUM") as otp,
                tc.tile_pool(name="pu", bufs=8) as pupool,
                tc.tile_pool(name="dd", bufs=10) as ddpool,
                tc.tile_pool(name="sm", bufs=16) as smpool,
                tc.tile_pool(name="pts", bufs=3) as ptspool,
                tc.tile_pool(name="wop", bufs=1) as wopool,
                tc.tile_pool(name="co", bufs=2) as copool,
            ):
                wo_sb = wopool.tile([P, HPC, DM], F32R)
                nc.sync.dma_start(wo_sb[:], wo.rearrange("(h p) m -> p h m", p=P))

                for qc in range(NSC):
                    nfull = qc
                    kmax = 4 * qc + 3
                    pu_l = {}
                    d_l = {}
                    # interleave the two heads' per-qb softmax chains so one
                    # head's matmuls fill the other's reduce/exp latency
                    for qbi in range(4):
                        for h in range(HPC):
                            qb = 4 * qc + qbi
                            r = qb % 4
                            qsl = slice(qb * P, (qb + 1) * P)
                            # --- bf16 max pre-pass: approximate row max ---
                            mparts = smpool.tile([P, 8], F32, tag="mp", name="mparts")
                            for kc in range(nfull + 1):
                                w = 512 if kc < nfull else (r + 1) * P
                                ksl = slice(kc * 512, kc * 512 + w)
                                mx = mxp.tile([P, 512], F32, name="mx")
                                nc.tensor.matmul(mx[:, :w], lhsT=q_hb[:, h, qsl],
                                                 rhs=k_hb[:, ksl], start=True, stop=True)
                                if kc == nfull:
                                    nc.vector.tensor_add(
                                        mx[:, r * P:(r + 1) * P],
                                        mx[:, r * P:(r + 1) * P], triu_sb[:])
                                nc.vector.tensor_reduce(
                                    mparts[:, kc:kc + 1], mx[:, :w], axis=AX, op=ALU.max)
                            negm = smpool.tile([P, 1], F32, tag="negm", name="negm")
                            nc.vector.tensor_reduce(
                                negm[:], mparts[:, :nfull + 1], axis=AX, op=ALU.max,
                                negate=True)
                            bias_t = smpool.tile([P, 1], F32, tag="bias", name="bias_t")
                            nc.vector.tensor_scalar_mul(bias_t[:], negm[:], SCALING)
                            # --- fp32r scores; exp immediately, no reduce ---
                            pu = pupool.tile([P, S], BF16, tag="pu", name="pu")
                            zparts = smpool.tile([P, 8], F32, tag="zp", name="zparts")
                            for kc in range(nfull + 1):
                                w = 512 if kc < nfull else (r + 1) * P
                                ksl = slice(kc * 512, kc * 512 + w)
                                ps = scp.tile([P, 512], F32, name="ps")
                                nc.tensor.matmul(
                                    ps[:, :w], lhsT=q_hi[:, h, qsl], rhs=k_hi[:, ksl],
                                    start=True, stop=True)
                                if kc == nfull:
                                    nc.vector.tensor_add(
                                        ps[:, r * P:(r + 1) * P],
                                        ps[:, r * P:(r + 1) * P], triu_sb[:])
                                nc.scalar.activation(
                                    pu[:, kc * 512:kc * 512 + w], ps[:, :w], AF.Exp,
                                    scale=SCALING, bias=bias_t[:],
                                    accum_out=zparts[:, kc:kc + 1])
                            zsum = smpool.tile([P, 1], F32, tag="zs", name="zsum")
                            nc.vector.tensor_reduce(
                                zsum[:], zparts[:, :nfull + 1], axis=AX, op=ALU.add)
                            rz = smpool.tile([P, 1], F32, tag="rz", name="rz")
                            nc.vector.reciprocal_approx_fast(rz[:], zsum[:])
                            dmat = ddpool.tile([P, P], BF16, tag="dm", name="dmat")
                            nc.vector.tensor_scalar_mul(dmat[:], identb_sb[:], rz[:])
                            pu_l[(h, qb)] = pu
                            d_l[(h, qb)] = dmat
                    for h in range(HPC):
                        # --- PuT (normalized) + AV accumulation ---
                        ot_ps = otp.tile([P, 512], F32, name="ot_ps")
                        for kb in range(kmax + 1):
                            putp = ptp.tile([P, 512], F32, name="putp")
                            i0 = max(kb - 4 * qc, 0)
                            for j in range(i0, 4):
                                qb = 4 * qc + j
                                nc.tensor.matmul(
                                    putp[:, j * P:(j + 1) * P],
                                    lhsT=pu_l[(h, qb)][:, kb * P:(kb + 1) * P],
                                    rhs=d_l[(h, qb)][:],
                                    start=True, stop=True)
                            # q-positions before i0*P don't attend to block kb:
                            # copy/accumulate only the live column range
                            puts = ptspool.tile([P, 512], BF16, name="puts")
                            if kb % 2 == 0:
                                nc.vector.tensor_copy(puts[:, i0 * P:], putp[:, i0 * P:])
                            else:
                                nc.scalar.copy(puts[:, i0 * P:], putp[:, i0 * P:])
                            nc.tensor.matmul(
                                ot_ps[:, i0 * P:], lhsT=v_r[:, kb, :],
                                rhs=puts[:, i0 * P:],
                                start=(kb == 0), stop=(kb == kmax))
                        csl = slice(qc * 512, (qc + 1) * 512)
                        nc.vector.tensor_mul(gated_r[:, h, csl], ot_ps[:],
                                             sig_r[:, h, csl])
                    # --- Wo partial for this q-chunk (both heads ready);
                    # pso ping-pongs the two dead PuT banks so each tile's
                    # PSUM->SBUF copy overlaps the next tile's matmuls ---
                    for sb in range(4 * qc, 4 * qc + 4):
                        cpo = copool.tile([P, NSC, 512], F32, name="cpo")
                        for dc in range(NSC):
                            pso = ptp.tile([P, 512], F32, name="putp")
                            for h in range(HPC):
                                nc.tensor.matmul(
                                    pso[:],
                                    lhsT=gated_r[:, h, sb * P:(sb + 1) * P],
                                    rhs=wo_sb[:, h, dc * 512:(dc + 1) * 512],
                                    start=(h == 0), stop=(h == HPC - 1))
                            if dc % 2 == 0:
                                nc.vector.tensor_copy(cpo[:, dc, :], pso[:])
                            else:
                                nc.scalar.copy(cpo[:, dc, :], pso[:])
                        nc.sync.dma_start(
                            out[sb * P:(sb + 1) * P, :],
                            cpo[:].rearrange("p dc m -> p (dc m)"))

    nc.compile()
    return nc


def _host_inputs(hidden_states, cos, sin, Wq, Wk, Wv, Wo, q_norm_w, k_norm_w):
    import ml_dtypes
    hs = np.asarray(hidden_states, dtype=np.float32).reshape(S, DM)
    hsT = _round_fp32r(np.ascontiguousarray(hs.T))
    cosT = np.ascontiguousarray(np.asarray(cos, np.float32).T)
    sinT = np.ascontiguousarray(np.asarray(sin, np.float32).T)
    sinTs = sinT.copy()
    sinTs[:D // 2] = -sinTs[:D // 2]     # sign of rotate-half folded into sin
    identb = np.eye(P, dtype=np.float32).astype(ml_dtypes.bfloat16)
    oner = np.ones((1, P), np.float32)
    triu = np.triu(np.full((P, P), -1e9, np.float32), 1)
    qw = np.asarray(q_norm_w, np.float32).reshape(D)
    kw = np.asarray(k_norm_w, np.float32).reshape(D)
    # fold norm weights into the projection columns; sumsq then needs 1/w^2
    Wq = np.asarray(Wq, np.float32).copy()
    for h in range(16):
        Wq[:, h * 2 * D:h * 2 * D + D] *= qw[None, :]
    Wk = np.asarray(Wk, np.float32) * np.tile(kw, 4)[None, :]
    Wv = np.asarray(Wv, np.float32)
    Wo = np.asarray(Wo, np.float32)
    wi2q = _round_fp32r((1.0 / (qw * qw)).reshape(P, 1))
    wi2k = _round_fp32r((1.0 / (kw * kw)).reshape(P, 1))
    maps = []
    for c in range(NCORES):
        heads = [2 * c, 2 * c + 1]
        g = c // 2
        wq_c = np.concatenate([Wq[:, h * 2 * D:h * 2 * D + D] for h in heads], axis=1)
        wg_c = np.concatenate([Wq[:, h * 2 * D + D:(h + 1) * 2 * D] for h in heads], axis=1)
        maps.append({
            "hsT": hsT,
            "wq": _round_fp32r(wq_c),
            "wk": _round_fp32r(Wk[:, g * D:(g + 1) * D]),
            "wg": _round_fp32r(wg_c),
            "wv": _round_fp32r(Wv[:, g * D:(g + 1) * D]),
            "wo": _round_fp32r(Wo[c * 2 * D:(c + 1) * 2 * D, :]),
            "cosT": cosT, "sinTs": sinTs,
            "wi2q": wi2q, "wi2k": wi2k,
            "identb": identb, "oner": oner, "triu": triu,
        })
    return maps


def kernel(**inputs):
    from concourse.bass_utils import run_bass_kernel_spmd

    if "nc" not in _cache:
        _cache["nc"] = _build_nc()
    nc = _cache["nc"]
    maps = _host_inputs(
        inputs["hidden_states"], inputs["cos"], inputs["sin"],
        inputs["Wq"], inputs["Wk"], inputs["Wv"], inputs["Wo"],
        inputs["q_norm_w"], inputs["k_norm_w"])
    res = run_bass_kernel_spmd(nc, maps, list(range(NCORES)))
    total = np.zeros((S, DM), np.float64)
    for r in res.results:
        total += r["out"].astype(np.float64)
    return total.astype(np.float32).reshape(1, S, DM)
